# revision 6
# baseline (speedup 1.0000x reference)
"""Bass/Tile TRN2 kernel for the binarized MLP (nn_Net_33569464385811).

Computation (reference):
    xb = sign(x);  h = xb @ sign(W1).T            # [8192, 384]
    BN (training stats over batch), hardtanh, sign
    out = sign(hn) @ sign(W4).T                   # [8192, 10]

Strategy:
  - Data parallel over batch: core c gets rows [c*1024, (c+1)*1024).
  - Host-side layout prep only: transpose x shard to [16384, 1024] (k-major)
    and cast to bf16 (lossless for sign()), same for W1 -> W1T [16384, 384].
  - On device per core:
      sign() via DVE dual-op tensor_scalar (x>=0)-0.5 -> +-0.5 in bf16 (4x mode);
      matmul h^T[j, b] accumulated over 128 k-tiles in PSUM (values = h/4);
      batch stats (sum, sum of squares) via DVE reduces; 3KB AllReduce across
      the 8 cores; sign((h-mean)*invstd*gamma + beta) via ACT Sign; tiny
      second matmul; DMA out [1024, 10] f32.
"""

import os
import sys
from contextlib import ExitStack

import numpy as np

sys.path.insert(0, "/opt/trn_rl_repo")

import ml_dtypes

import concourse.bass as bass
import concourse.tile as tile
from concourse import bacc, mybir
from concourse.alu_op_type import AluOpType
from concourse.bass_utils import run_bass_kernel_spmd

P = 128
NCORES = 8
BATCH = 8192
B = BATCH // NCORES          # per-core batch (1024)
KD = 16384                   # input features (contraction)
H = 384                      # hidden
O = 10                       # output classes
KS = 8                       # k-subtiles (of 128) per DMA chunk
NKC = KD // (P * KS)         # 16 chunks
NKT = KD // P                # 128 k-tiles
NJT = H // P                 # 3 hidden partition-tiles
NBT = B // P                 # 8 batch tiles per core
NC2 = 2                      # b-chunks of 512 for psum free dim
BN_EPS = 1e-5

F32 = mybir.dt.float32
BF16 = mybir.dt.bfloat16
AF = mybir.ActivationFunctionType


def build_kernel():
    nc = bacc.Bacc(
        "TRN2",
        target_bir_lowering=False,
        debug=False,
        num_devices=NCORES,
    )

    xt = nc.dram_tensor("xt", [KD, B], BF16, kind="ExternalInput")
    w1t = nc.dram_tensor("w1t", [KD, H], BF16, kind="ExternalInput")
    w4t = nc.dram_tensor("w4t", [H, O], F32, kind="ExternalInput")
    gam = nc.dram_tensor("gam", [H, 1], F32, kind="ExternalInput")
    bet = nc.dram_tensor("bet", [H, 1], F32, kind="ExternalInput")
    out = nc.dram_tensor("out", [B, O], F32, kind="ExternalOutput")

    # collective bounce buffers (3 sums + 3 sumsqs packed in 8 cols)
    cc_in = nc.dram_tensor("cc_in", [P, 8], F32)
    cc_out = nc.dram_tensor("cc_out", [P, 8], F32, addr_space="Shared")

    with tile.TileContext(nc) as tc, ExitStack() as ctx:
        const = ctx.enter_context(tc.tile_pool(name="const", bufs=1))
        xpool = ctx.enter_context(tc.tile_pool(name="x", bufs=3))
        xbpool = ctx.enter_context(tc.tile_pool(name="xb", bufs=3))
        w1pool = ctx.enter_context(tc.tile_pool(name="w1", bufs=2))
        w1bpool = ctx.enter_context(tc.tile_pool(name="w1b", bufs=2))
        hpool = ctx.enter_context(tc.tile_pool(name="h", bufs=1))
        spool = ctx.enter_context(tc.tile_pool(name="stats", bufs=1))
        tpool = ctx.enter_context(tc.tile_pool(name="tmp", bufs=2))
        opool = ctx.enter_context(tc.tile_pool(name="o", bufs=1))
        psum_h = ctx.enter_context(tc.tile_pool(name="psum_h", bufs=1, space="PSUM"))
        psum_o = ctx.enter_context(tc.tile_pool(name="psum_o", bufs=2, space="PSUM"))

        # --- small constants ---
        gam_sb = const.tile([P, NJT], F32, tag="gam")
        nc.sync.dma_start(
            gam_sb[:].unsqueeze(2), gam.rearrange("(t p) o -> p t o", p=P)
        )
        bet_sb = const.tile([P, NJT], F32, tag="bet")
        nc.sync.dma_start(
            bet_sb[:].unsqueeze(2), bet.rearrange("(t p) o -> p t o", p=P)
        )
        w4s = const.tile([P, NJT, O], F32, tag="w4s")
        nc.sync.dma_start(w4s, w4t.rearrange("(t p) o -> p t o", p=P))
        w4b = const.tile([P, NJT, O], BF16, tag="w4b")
        nc.scalar.activation(w4b, w4s, AF.Sign)

        # --- h^T accumulation in PSUM: 3 j-tiles x 2 chunks of 512 ---
        ph = [
            [psum_h.tile([P, 512], F32, tag=f"ph{j}_{c}", name=f"ph{j}_{c}") for c in range(NC2)]
            for j in range(NJT)
        ]

        for kc in range(NKC):
            r0 = kc * KS * P
            r1 = (kc + 1) * KS * P
            xs = xpool.tile([P, KS, B], BF16, tag="xs")
            nc.sync.dma_start(xs, xt[r0:r1, :].rearrange("(s p) b -> p s b", p=P))
            w1s = w1pool.tile([P, KS, H], BF16, tag="w1s")
            nc.sync.dma_start(w1s, w1t[r0:r1, :].rearrange("(s p) h -> p s h", p=P))

            # binarize to +-0.5 (bf16, DVE 4x mode): (v >= 0) - 0.5
            xb = xbpool.tile([P, KS, B], BF16, tag="xbt")
            nc.vector.tensor_scalar(
                xb[:].rearrange("p s b -> p (s b)"),
                xs[:].rearrange("p s b -> p (s b)"),
                0.0,
                0.5,
                AluOpType.is_ge,
                AluOpType.subtract,
            )
            w1b = w1bpool.tile([P, KS, H], BF16, tag="w1bt")
            nc.vector.tensor_scalar(
                w1b[:].rearrange("p s h -> p (s h)"),
                w1s[:].rearrange("p s h -> p (s h)"),
                0.0,
                0.5,
                AluOpType.is_ge,
                AluOpType.subtract,
            )

            for s in range(KS):
                kt = kc * KS + s
                for j in range(NJT):
                    lhsT = w1b[:, s : s + 1, j * P : (j + 1) * P]
                    for c in range(NC2):
                        nc.tensor.matmul(
                            ph[j][c],
                            lhsT=lhsT,
                            rhs=xb[:, s : s + 1, c * 512 : (c + 1) * 512],
                            start=(kt == 0),
                            stop=(kt == NKT - 1),
                        )

        # --- local stats: cols 0..2 = sum(h), 3..5 = sum(h^2) ---
        stats = spool.tile([P, 8], F32, tag="stats")
        nc.vector.memset(stats, 0.0)
        h_sb = hpool.tile([P, NJT * B], F32, tag="h")
        for j in range(NJT):
            for c in range(NC2):
                # psum holds h/4 (both matmul operands were +-0.5): scale by 4
                nc.scalar.activation(
                    h_sb[:, j * B + c * 512 : j * B + (c + 1) * 512],
                    ph[j][c],
                    AF.Copy,
                    scale=4.0,
                )
            hj = h_sb[:, j * B : (j + 1) * B]
            nc.vector.tensor_reduce(
                stats[:, j : j + 1], hj, axis=mybir.AxisListType.X, op=AluOpType.add
            )
            # NOTE: tensor_tensor_reduce (raw-ISA op) crashes this runtime;
            # use plain mult + reduce instead.
            sq = tpool.tile([P, B], F32, tag="sq")
            nc.vector.tensor_tensor(sq, hj, hj, AluOpType.mult)
            nc.vector.tensor_reduce(
                stats[:, 3 + j : 4 + j], sq, axis=mybir.AxisListType.X, op=AluOpType.add
            )

        # --- all-reduce stats across the 8 cores ---
        nc.sync.dma_start(cc_in.ap(), stats)
        nc.gpsimd.collective_compute(
            "AllReduce",
            AluOpType.add,
            replica_groups=[list(range(NCORES))],
            ins=[cc_in.ap().opt()],
            outs=[cc_out.ap().opt()],
        )
        gst = spool.tile([P, 8], F32, tag="gst")
        nc.sync.dma_start(gst, cc_out.ap())

        # --- mean / var / scale ---
        mean = spool.tile([P, NJT], F32, tag="mean")
        nc.vector.tensor_scalar(mean, gst[:, 0:NJT], 1.0 / BATCH, None, AluOpType.mult)
        esq = spool.tile([P, NJT], F32, tag="esq")
        nc.vector.tensor_scalar(
            esq, gst[:, 3 : 3 + NJT], 1.0 / BATCH, None, AluOpType.mult
        )
        eps_sb = spool.tile([P, 1], F32, tag="eps")
        nc.vector.memset(eps_sb, BN_EPS)
        var = spool.tile([P, NJT], F32, tag="var")
        nc.vector.tensor_tensor(var, mean, mean, AluOpType.mult)
        nc.vector.tensor_tensor(var, esq, var, AluOpType.subtract)
        std = spool.tile([P, NJT], F32, tag="std")
        nc.scalar.activation(std, var, AF.Sqrt, bias=eps_sb[:])
        invstd = spool.tile([P, NJT], F32, tag="invstd")
        nc.vector.reciprocal(invstd, std)
        scl = spool.tile([P, NJT], F32, tag="scl")
        nc.vector.tensor_tensor(scl, invstd, gam_sb, AluOpType.mult)

        # --- apply BN + sign:  sgn = Sign((h - mean) * scl + beta) ---
        sgn = opool.tile([P, NJT * B], BF16, tag="sgn")
        for j in range(NJT):
            tmp = tpool.tile([P, B], F32, tag="bn")
            nc.vector.tensor_scalar(
                tmp,
                h_sb[:, j * B : (j + 1) * B],
                mean[:, j : j + 1],
                None,
                AluOpType.subtract,
            )
            nc.scalar.activation(
                sgn[:, j * B : (j + 1) * B],
                tmp,
                AF.Sign,
                bias=bet_sb[:, j : j + 1],
                scale=scl[:, j : j + 1],
            )

        # --- second matmul: out[b, o] = sum_j sgn[j, b] * w4b[j, o] ---
        out_sb = opool.tile([P, NBT, O], F32, tag="osb")
        for bt in range(NBT):
            po = psum_o.tile([P, O], F32, tag="po")
            for j in range(NJT):
                nc.tensor.matmul(
                    po,
                    lhsT=sgn[:, j * B + bt * P : j * B + (bt + 1) * P],
                    rhs=w4b[:, j : j + 1, :],
                    start=(j == 0),
                    stop=(j == NJT - 1),
                )
            nc.scalar.activation(out_sb[:, bt : bt + 1, :], po, AF.Copy)
        nc.sync.dma_start(out.rearrange("(t p) o -> p t o", p=P), out_sb)

    nc.compile()
    return nc


_NC_CACHE = None


def kernel(x, W1, gamma, beta, W4):
    global _NC_CACHE
    x = np.asarray(x)
    W1 = np.asarray(W1)
    gamma = np.asarray(gamma)
    beta = np.asarray(beta)
    W4 = np.asarray(W4)

    # host-side layout prep (lossless for the sign() computation)
    w1t = np.ascontiguousarray(W1.T.astype(ml_dtypes.bfloat16))
    w4t = np.ascontiguousarray(W4.T.astype(np.float32))
    g = np.ascontiguousarray(gamma.reshape(H, 1).astype(np.float32))
    b = np.ascontiguousarray(beta.reshape(H, 1).astype(np.float32))

    in_maps = []
    for c in range(NCORES):
        xs = x[c * B : (c + 1) * B].astype(ml_dtypes.bfloat16)
        xtc = np.ascontiguousarray(xs.T)  # [16384, 1024] bf16
        in_maps.append({"xt": xtc, "w1t": w1t, "w4t": w4t, "gam": g, "bet": b})

    if _NC_CACHE is None:
        _NC_CACHE = build_kernel()
    nc = _NC_CACHE

    res = run_bass_kernel_spmd(nc, in_maps, list(range(NCORES)))
    outs = [np.asarray(r["out"], dtype=np.float32) for r in res.results]
    return np.concatenate(outs, axis=0)


if __name__ == "__main__":
    build_kernel()
    print("kernel built ok")
